# revision 1
# baseline (speedup 1.0000x reference)
"""CRF loss (forward-algorithm partition + gold energy) on 8 TRN2 NeuronCores.

Strategy (data-parallel over batch, per the sharding hint):
  - batch 64 -> 8 cores x 8 local batches.
  - Forward recurrence kept in the *linear* domain: state q[t', b] with
    partition[b, t'] = ln q[t', b] + sum_k ln(m_k[b]).  One step is
    q <- E_b^T q per local batch (E = exp(scores[s,b])), computed as 8 tiny
    PE matvecs against bf16 E tiles produced by one big ScalarE exp per
    chunk of timesteps.  exp/log of the textbook logsumexp cancel between
    steps, so ScalarE only exponentiates each score element once.
  - Every 8 steps the state is renormalized by its column sum (computed with
    a ones-vector matmul; scaling broadcast across partitions with a rank-1
    matmul), and the sum is stashed; all logs are deferred to two ScalarE
    Ln instructions at the very end.
  - Gold-path energy: indirect-DMA element gather with host-precomputed flat
    indices, masked multiply-reduce on VectorE.
  - Per-core partials (final ln q, stashed renorm sums' logs, gold partial)
    are combined into the scalar loss on the host.
"""

import numpy as np

import concourse.bacc as bacc
import concourse.bass as bass
import concourse.mybir as mybir
import concourse.tile as tile
from concourse import bass_utils

S = 256
B = 64
T = 128
NCORES = 8
BL = B // NCORES  # 8 local batches per core
START_TAG = 126
END_TAG = 127
CHUNK = 4  # timesteps per score DMA + exp instruction
RENORM_START = 6
RENORM_EVERY = 8

f32 = mybir.dt.float32
bf16 = mybir.dt.bfloat16
i32 = mybir.dt.int32
u8 = mybir.dt.uint8
Exp = mybir.ActivationFunctionType.Exp
Ln = mybir.ActivationFunctionType.Ln
Alu = mybir.AluOpType


def renorm_steps(n_steps):
    return [s for s in range(RENORM_START, n_steps - 1, RENORM_EVERY)]


def build(n_steps=S):
    """Build + compile the SPMD kernel for one core's batch shard."""
    nrn = renorm_steps(n_steps)
    n_gather = -(-n_steps * BL // 128)  # gather columns (2048 idx -> [128, 16])
    nc = bacc.Bacc(
        "TRN2", target_bir_lowering=False, debug=False, num_devices=NCORES
    )
    sc = nc.dram_tensor("scores", [n_steps, T, BL, T], f32, kind="ExternalInput")
    p0 = nc.dram_tensor("p0t", [T, BL], f32, kind="ExternalInput").ap()
    mk = nc.dram_tensor("masks", [T, n_steps * BL], u8, kind="ExternalInput").ap()
    gi = nc.dram_tensor("tg_idx", [128, n_gather], i32, kind="ExternalInput").ap()
    gm = nc.dram_tensor("tg_msk", [128, n_gather], f32, kind="ExternalInput").ap()
    o_logq = nc.dram_tensor("out_logq", [T, BL], f32, kind="ExternalOutput").ap()
    o_tg = nc.dram_tensor("out_tg", [128, 1], f32, kind="ExternalOutput").ap()
    o_lnm = None
    if nrn:
        o_lnm = nc.dram_tensor(
            "out_lnm", [1, len(nrn) * BL], f32, kind="ExternalOutput"
        ).ap()

    with tile.TileContext(nc) as tc:
        _body(nc, tc, sc, p0, mk, gi, gm, o_logq, o_tg, o_lnm, n_steps, nrn)
    nc.compile()
    return nc


def _body(nc, tc, sc, p0, mk, gi, gm, o_logq, o_tg, o_lnm, n_steps, nrn):
    import os
    from contextlib import ExitStack

    nogather = os.environ.get("K_NOGATHER")
    nomasks = os.environ.get("K_NOMASKS")
    norenorm = os.environ.get("K_NORENORM")
    noexp = os.environ.get("K_NOEXP")
    nomm = os.environ.get("K_NOMM")
    repeat = int(os.environ.get("K_REPEAT", "1"))

    n_gather = gi.shape[1]
    sc_ap = sc.ap()

    with ExitStack() as ctx:
        const = ctx.enter_context(tc.tile_pool(name="const", bufs=1))
        spool = ctx.enter_context(tc.tile_pool(name="spool", bufs=3))
        epool = ctx.enter_context(tc.tile_pool(name="epool", bufs=3))
        vpool = ctx.enter_context(tc.tile_pool(name="vpool", bufs=4, space="PSUM"))
        rpool = ctx.enter_context(tc.tile_pool(name="rpool", bufs=2, space="PSUM"))
        small = ctx.enter_context(tc.tile_pool(name="small", bufs=2))

        # ---- constants & persistent state ----
        ones_col = const.tile([128, 1], bf16)
        nc.vector.memset(ones_col[:], 1.0)
        ones_row = const.tile([1, 128], f32)
        nc.vector.memset(ones_row[:], 1.0)
        q = const.tile([128, BL], bf16)  # recurrence state
        mbuf = None
        if nrn and not nomm:
            mbuf = const.tile([1, len(nrn) * BL], f32)  # stashed renorm sums
        masks_sb = const.tile([128, n_steps * BL], u8)
        nc.sync.dma_start(out=masks_sb[:], in_=mk[:])

        # ---- init: q = exp(scores[0, :, START_TAG, :]^T) ----
        p0_sb = small.tile([128, BL], f32)
        nc.sync.dma_start(out=p0_sb[:], in_=p0[:])

        # ---- gold energy gather (independent of the recurrence) ----
        if nogather:
            tgz = const.tile([128, 1], f32)
            nc.vector.memset(tgz[:], 0.0)
            nc.sync.dma_start(out=o_tg[:], in_=tgz[:])
        gidx = const.tile([128, n_gather], i32)
        if not nogather:
            nc.sync.dma_start(out=gidx[:], in_=gi[:])
        if not nogather:
            gmask = const.tile([128, n_gather], f32)
            nc.sync.dma_start(out=gmask[:], in_=gm[:])
            gath = const.tile([128, n_gather], f32)
            n_elem = n_steps * BL * T * T
            sc_flat = bass.AP(tensor=sc, offset=0, ap=[[1, n_elem], [1, 1]])
            for j in range(n_gather):
                nc.gpsimd.indirect_dma_start(
                    out=gath[:, j : j + 1],
                    out_offset=None,
                    in_=sc_flat,
                    in_offset=bass.IndirectOffsetOnAxis(ap=gidx[:, j : j + 1], axis=0),
                )
            prod = const.tile([128, n_gather], f32)
            tgc = const.tile([128, 1], f32)
            nc.vector.tensor_tensor(
                out=prod[:], in0=gath[:], in1=gmask[:], op=Alu.mult
            )
            nc.vector.reduce_sum(
                out=tgc[:], in_=prod[:], axis=mybir.AxisListType.X
            )
            nc.sync.dma_start(out=o_tg[:], in_=tgc[:])

        # ---- main recurrence over timesteps 1..n_steps-1 ----
        nrn_set = set(nrn)
        for rep in range(repeat):
            nc.scalar.activation(out=q[:], in_=p0_sb[:], func=Exp)
            k_renorm = 0
            s = 1
            while s < n_steps:
                hi = min(s + CHUNK, n_steps)
                nsub = hi - s
                # stream scores[s:hi] as [t, (s b u)] and exponentiate once
                sc_tile = spool.tile([128, nsub * BL * T], f32, tag="sc")
                nc.sync.dma_start(
                    out=sc_tile[:],
                    in_=sc_ap[s:hi].rearrange("s t b u -> t s b u"),
                )
                if noexp:
                    e_tile = sc_tile.bitcast(bf16)[:, : nsub * BL * T]
                else:
                    e_tile = epool.tile([128, nsub * BL * T], bf16, tag="e")
                    nc.scalar.activation(out=e_tile[:], in_=sc_tile[:], func=Exp)
                for sl in range(nsub):
                    step = s + sl
                    if nomm:
                        continue
                    v = vpool.tile([128, BL], f32, tag="v")
                    for b in range(BL):
                        off = (sl * BL + b) * T
                        nc.tensor.matmul(
                            out=v[:, b : b + 1],
                            lhsT=e_tile[:, off : off + T],
                            rhs=q[:, b : b + 1],
                            start=True,
                            stop=True,
                        )
                    # q <- v where mask_for_padding[step] else q
                    if nomasks:
                        nc.vector.tensor_copy(out=q[:], in_=v[:])
                    else:
                        nc.vector.copy_predicated(
                            out=q[:],
                            mask=masks_sb[:, step * BL : (step + 1) * BL],
                            data=v[:],
                        )
                    if step in nrn_set and not norenorm:
                        ssum = rpool.tile([1, BL], f32, tag="sum")
                        nc.tensor.matmul(
                            out=ssum[:],
                            lhsT=ones_col[:],
                            rhs=q[:],
                            start=True,
                            stop=True,
                        )
                        nc.vector.tensor_copy(
                            out=mbuf[:, k_renorm * BL : (k_renorm + 1) * BL],
                            in_=ssum[:],
                        )
                        r_row = small.tile([1, BL], f32, tag="rrow")
                        nc.vector.reciprocal(out=r_row[:], in_=ssum[:])
                        r_bc = rpool.tile([128, BL], f32, tag="rbc")
                        nc.tensor.matmul(
                            out=r_bc[:],
                            lhsT=ones_row[:],
                            rhs=r_row[:],
                            start=True,
                            stop=True,
                        )
                        nc.vector.tensor_tensor(
                            out=q[:], in0=q[:], in1=r_bc[:], op=Alu.mult
                        )
                        k_renorm += 1
                s = hi

        # ---- finalize ----
        logq = small.tile([128, BL], f32, tag="logq")
        nc.scalar.activation(out=logq[:], in_=q[:], func=Ln)
        nc.sync.dma_start(out=o_logq[:], in_=logq[:])
        if nrn:
            lnm_t = small.tile([1, len(nrn) * BL], f32, tag="lnm")
            if mbuf is None:
                nc.vector.memset(lnm_t[:], 0.0)
            else:
                nc.scalar.activation(out=lnm_t[:], in_=mbuf[:], func=Ln)
            nc.sync.dma_start(out=o_lnm[:], in_=lnm_t[:])


def make_in_maps(scores, target, mask_gold, mask_pad, n_steps=S):
    """Host-side sharding/preprocessing -> per-core input dicts."""
    scores = np.asarray(scores, dtype=np.float32)
    target = np.asarray(target).astype(np.int64)
    mg = np.asarray(mask_gold).astype(np.float32)
    mp = np.asarray(mask_pad).astype(np.float32)
    n_gather = -(-n_steps * BL // 128)
    in_maps = []
    for c in range(NCORES):
        b0 = c * BL
        sc_c = np.ascontiguousarray(
            scores[:n_steps, b0 : b0 + BL].transpose(0, 2, 1, 3)
        )
        p0_c = np.ascontiguousarray(scores[0, b0 : b0 + BL, START_TAG, :].T)
        mrow = mp[:n_steps, b0 : b0 + BL].reshape(-1)
        mk_c = np.ascontiguousarray(
            np.broadcast_to(mrow[None, :], (128, n_steps * BL))
        ).astype(np.uint8)
        tgt = target[:n_steps, b0 : b0 + BL, 0]
        tfrom = tgt // T
        tto = tgt % T
        sidx = (
            (
                (np.arange(n_steps, dtype=np.int64)[:, None] * T + tfrom) * BL
                + np.arange(BL, dtype=np.int64)[None, :]
            )
            * T
            + tto
        ).reshape(-1)
        gmv = mg[:n_steps, b0 : b0 + BL].reshape(-1)
        pad = n_gather * 128 - sidx.shape[0]
        if pad:
            sidx = np.concatenate([sidx, np.zeros(pad, dtype=np.int64)])
            gmv = np.concatenate([gmv, np.zeros(pad, dtype=np.float32)])
        gi_c = np.ascontiguousarray(
            sidx.reshape(n_gather, 128).T.astype(np.int32)
        )
        gm_c = np.ascontiguousarray(gmv.reshape(n_gather, 128).T)
        in_maps.append(
            {
                "scores": sc_c,
                "p0t": p0_c,
                "masks": mk_c,
                "tg_idx": gi_c,
                "tg_msk": gm_c,
            }
        )
    return in_maps


def combine(results, n_steps=S):
    """Host-side reduction of per-core partials -> scalar loss."""
    part = 0.0
    tg = 0.0
    for r in results:
        part += float(r["out_logq"][END_TAG, :].sum(dtype=np.float64))
        if "out_lnm" in r:
            part += float(r["out_lnm"].sum(dtype=np.float64))
        tg += float(r["out_tg"].sum(dtype=np.float64))
    return np.float32((part - tg) / B)


_NC_CACHE = {}


def kernel(scores, target, mask_for_gold, mask_for_padding):
    if "nc" not in _NC_CACHE:
        _NC_CACHE["nc"] = build(S)
    nc = _NC_CACHE["nc"]
    in_maps = make_in_maps(scores, target, mask_for_gold, mask_for_padding, S)
    res = bass_utils.run_bass_kernel_spmd(
        nc, in_maps, core_ids=list(range(NCORES))
    )
    return combine(res.results, S)



# revision 4
# speedup vs baseline: 1.0653x; 1.0653x over previous
"""CRF loss (forward-algorithm partition + gold energy) on 8 TRN2 NeuronCores.

Strategy (data-parallel over batch; 64 batches -> 8 cores x 8 local):
  - Linear-domain recurrence q <- E^T q per local batch (E = exp(scores)),
    one 128x128xBL-matvec group per timestep on the PE.  exp/log of the
    textbook logsumexp cancel between steps so each score element is
    exponentiated exactly once.
  - A constant shift c = ln(128*sqrt(e)) is folded into every exponential
    (ACT bias / Schraudolph constant) so q stays O(1) for all 255 steps:
    no renormalization is needed at all.  The host adds 256*c per batch
    when assembling the final loss.
  - exp work is split across TWO engines: batches 4..7 stream as fp8e4
    and go through ScalarE's exp LUT; batches 0..3 stream as bf16 and go
    through a one-instruction Schraudolph exp on VectorE (tensor_scalar
    mult+add producing bf16 *bit patterns* as int16, 4x perf mode, with a
    debias constant).  This roughly halves both HBM traffic (1.5 B/elem
    avg) and ScalarE busy time vs an all-f32 ScalarE design.
  - mask_for_padding is folded into the scores on the host: blocks with
    mask==0 are replaced by a log-domain identity (0 diag / -30 off-diag),
    which exponentiates to e^-c * I and leaves the recurrence state
    unchanged up to the uniformly-counted shift.  (The harness always
    passes an all-ones padding mask, so this rewrites nothing.)
  - Scores are stored partition-major [tag_from, step, batch, tag_to] so
    every chunk DMA is one fully contiguous run per partition.
  - Gold-path energy: indirect-DMA element gathers with host-precomputed
    flat indices (one set per dtype tensor), masked multiply-reduce on
    VectorE.
"""

import math
import os

import numpy as np

import concourse.bacc as bacc
import concourse.bass as bass
import concourse.mybir as mybir
import concourse.tile as tile
from concourse import bass_utils

S = 256
B = 64
T = 128
NCORES = 8
BL = B // NCORES  # 8 local batches per core
BH = BL // 2  # 4 batches per exp-engine half
START_TAG = 126
END_TAG = 127
CHUNK = 16  # timesteps per score DMA

# constant shift folded into every exp so q stays O(1) without renorm
C_SHIFT = math.log(128.0) + 0.5
# Schraudolph exp-as-bf16-bits constants: bits = round(x*A + BCONST)
A_SCH = 128.0 / math.log(2.0)  # 184.6644
# 16256 = 127<<7 (bf16 exponent bias), minus c in bits, minus mean-error debias
B_SCH = 16256.0 - C_SHIFT * A_SCH - (0.0397 / math.log(2.0)) * 128.0

f32 = mybir.dt.float32
bf16 = mybir.dt.bfloat16
fp8 = mybir.dt.float8e4
i16 = mybir.dt.int16
i32 = mybir.dt.int32
Exp = mybir.ActivationFunctionType.Exp
Alu = mybir.AluOpType

NG = S * BH // 128  # gather columns per dtype tensor (= 8)


def build(n_steps=S):
    """Build + compile the SPMD kernel for one core's batch shard."""
    nc = bacc.Bacc(
        "TRN2", target_bir_lowering=False, debug=False, num_devices=NCORES
    )
    # partition-major layouts: [tag_from, step, local_batch_half, tag_to]
    sc16 = nc.dram_tensor("sc16", [T, n_steps, BH, T], bf16, kind="ExternalInput")
    sc8 = nc.dram_tensor("sc8", [T, n_steps, BH, T], fp8, kind="ExternalInput")
    p0 = nc.dram_tensor("p0t", [T, BL], f32, kind="ExternalInput").ap()
    gi16 = nc.dram_tensor("g16_idx", [128, NG], i32, kind="ExternalInput").ap()
    gm16 = nc.dram_tensor("g16_msk", [128, NG], f32, kind="ExternalInput").ap()
    gi8 = nc.dram_tensor("g8_idx", [128, NG], i32, kind="ExternalInput").ap()
    gm8 = nc.dram_tensor("g8_msk", [128, NG], f32, kind="ExternalInput").ap()
    o_q = nc.dram_tensor("out_q", [T, BL], f32, kind="ExternalOutput").ap()
    o_tg = nc.dram_tensor("out_tg", [128, 1], f32, kind="ExternalOutput").ap()

    with tile.TileContext(nc) as tc:
        _body(nc, tc, sc16, sc8, p0, gi16, gm16, gi8, gm8, o_q, o_tg, n_steps)
    nc.compile()
    return nc


def _body(nc, tc, sc16, sc8, p0, gi16, gm16, gi8, gm8, o_q, o_tg, n_steps):
    from contextlib import ExitStack

    nogather = os.environ.get("K_NOGATHER")
    repeat = int(os.environ.get("K_REPEAT", "1"))

    sc16_ap = sc16.ap()
    sc8_ap = sc8.ap()

    with ExitStack() as ctx:
        const = ctx.enter_context(tc.tile_pool(name="const", bufs=1))
        s16p = ctx.enter_context(tc.tile_pool(name="s16p", bufs=3))
        s8p = ctx.enter_context(tc.tile_pool(name="s8p", bufs=3))
        epool = ctx.enter_context(tc.tile_pool(name="epool", bufs=3))
        vpool = ctx.enter_context(tc.tile_pool(name="vpool", bufs=4, space="PSUM"))
        small = ctx.enter_context(tc.tile_pool(name="small", bufs=2))

        # ---- persistent state (ping-pong) ----
        q0 = const.tile([128, BL], bf16)
        q1 = const.tile([128, BL], bf16)
        p0_sb = const.tile([128, BL], f32)
        nc.sync.dma_start(out=p0_sb[:], in_=p0[:])
        biasc = const.tile([128, 1], f32)
        nc.vector.memset(biasc[:], -C_SHIFT)

        # ---- gold energy gather (independent of the recurrence) ----
        if not nogather:
            gidx16 = const.tile([128, NG], i32)
            gidx8 = const.tile([128, NG], i32)
            gmsk = const.tile([128, 2 * NG], f32)
            nc.sync.dma_start(out=gidx16[:], in_=gi16[:])
            nc.sync.dma_start(out=gidx8[:], in_=gi8[:])
            nc.sync.dma_start(out=gmsk[:, :NG], in_=gm16[:])
            nc.sync.dma_start(out=gmsk[:, NG:], in_=gm8[:])
            gath16 = const.tile([128, NG], bf16)
            gath8 = const.tile([128, NG], fp8)
            n16 = T * n_steps * BH * T
            sc16_flat = bass.AP(tensor=sc16, offset=0, ap=[[1, n16], [1, 1]])
            sc8_flat = bass.AP(tensor=sc8, offset=0, ap=[[1, n16], [1, 1]])
            for j in range(NG):
                nc.gpsimd.indirect_dma_start(
                    out=gath16[:, j : j + 1],
                    out_offset=None,
                    in_=sc16_flat,
                    in_offset=bass.IndirectOffsetOnAxis(
                        ap=gidx16[:, j : j + 1], axis=0
                    ),
                )
                nc.gpsimd.indirect_dma_start(
                    out=gath8[:, j : j + 1],
                    out_offset=None,
                    in_=sc8_flat,
                    in_offset=bass.IndirectOffsetOnAxis(
                        ap=gidx8[:, j : j + 1], axis=0
                    ),
                )
            gcat = const.tile([128, 2 * NG], f32)
            nc.vector.tensor_copy(out=gcat[:, :NG], in_=gath16[:])
            nc.vector.tensor_copy(out=gcat[:, NG:], in_=gath8[:])
            prod = const.tile([128, 2 * NG], f32)
            nc.vector.tensor_tensor(
                out=prod[:], in0=gcat[:], in1=gmsk[:], op=Alu.mult
            )
            tgc = const.tile([128, 1], f32)
            nc.vector.reduce_sum(out=tgc[:], in_=prod[:], axis=mybir.AxisListType.X)
            nc.sync.dma_start(out=o_tg[:], in_=tgc[:])
        else:
            tgz = const.tile([128, 1], f32)
            nc.vector.memset(tgz[:], 0.0)
            nc.sync.dma_start(out=o_tg[:], in_=tgz[:])

        # ---- main recurrence over timesteps 1..n_steps-1 ----
        for rep in range(repeat):
            last_rep = rep == repeat - 1
            # q0 = exp(p0 - c)
            nc.scalar.activation(out=q0[:], in_=p0_sb[:], func=Exp, bias=biasc[:])
            s = 1
            while s < n_steps:
                hi = min(s + CHUNK, n_steps)
                ns = hi - s
                t16 = s16p.tile([128, ns, BH, T], bf16, tag="t16")
                nc.sync.dma_start(out=t16[:], in_=sc16_ap[:, s:hi])
                t8 = s8p.tile([128, ns, BH, T], fp8, tag="t8")
                nc.sync.dma_start(out=t8[:], in_=sc8_ap[:, s:hi])
                e = epool.tile([128, ns, BL, T], bf16, tag="e")
                # batches 0..3: Schraudolph exp on VectorE (bf16 bits via i16)
                nc.vector.tensor_scalar(
                    out=e.bitcast(i16)[:, :, 0:BH, :],
                    in0=t16[:],
                    scalar1=A_SCH,
                    scalar2=B_SCH,
                    op0=Alu.mult,
                    op1=Alu.add,
                )
                # batches 4..7: LUT exp on ScalarE
                nc.scalar.activation(
                    out=e[:, :, BH:BL, :], in_=t8[:], func=Exp, bias=biasc[:]
                )
                for sl in range(ns):
                    step = s + sl
                    qin = q0 if step % 2 == 1 else q1
                    qout = q1 if step % 2 == 1 else q0
                    v = vpool.tile([128, BL], f32, tag="v")
                    for b in range(BL):
                        nc.tensor.matmul(
                            out=v[:, b : b + 1],
                            lhsT=e[:, sl, b, :],
                            rhs=qin[:, b : b + 1],
                            start=True,
                            stop=True,
                        )
                    if step == n_steps - 1 and last_rep:
                        qf = small.tile([128, BL], f32, tag="qf")
                        nc.vector.tensor_copy(out=qf[:], in_=v[:])
                        nc.sync.dma_start(out=o_q[:], in_=qf[:])
                    else:
                        nc.vector.tensor_copy(out=qout[:], in_=v[:])
                s = hi


def make_in_maps(scores, target, mask_gold, mask_pad, n_steps=S):
    """Host-side sharding/layout -> per-core input dicts.

    Also returns (via module global) the host-side gather corrections for
    padding-masked blocks (exactly 0.0 for the all-ones padding mask).
    """
    global _HOST_TG_EXTRA
    scores = np.asarray(scores, dtype=np.float32)
    target = np.asarray(target).astype(np.int64)
    mg = np.asarray(mask_gold).astype(np.float32)
    mp = np.asarray(mask_pad).astype(np.int32)

    bf16_np = mybir.dt.np(bf16)
    fp8_np = mybir.dt.np(fp8)

    any_masked = not bool(np.all(mp[1:n_steps] == 1))
    if any_masked:
        diagpat = np.full((T, T), -30.0, dtype=np.float32)
        np.fill_diagonal(diagpat, 0.0)

    in_maps = []
    _HOST_TG_EXTRA = []
    for c in range(NCORES):
        b0 = c * BL
        blk = np.array(scores[:n_steps, b0 : b0 + BL])  # [S, BL, T, T] copy
        extra = 0.0
        if any_masked:
            ms, mb = np.nonzero(mp[1:n_steps, b0 : b0 + BL] == 0)
            if ms.size:
                blk[ms + 1, mb] = diagpat
        # [S, BL, T(from), T(to)] -> [T(from), S, BL, T(to)]
        tr = np.ascontiguousarray(blk.transpose(2, 0, 1, 3))
        sc16_c = tr[:, :, 0:BH, :].astype(bf16_np)
        sc8_c = np.ascontiguousarray(tr[:, :, BH:BL, :]).astype(fp8_np)
        p0_c = np.ascontiguousarray(scores[0, b0 : b0 + BL, START_TAG, :].T)

        tgt = target[:n_steps, b0 : b0 + BL, 0]
        tfrom = tgt // T
        tto = tgt % T
        gmv = np.array(mg[:n_steps, b0 : b0 + BL])
        if any_masked:
            # gathers that land in rewritten blocks: zero the device mask and
            # account for the original value on the host
            ms, mb = np.nonzero(mp[1:n_steps, b0 : b0 + BL] == 0)
            if ms.size:
                ss = ms + 1
                orig = scores[ss, b0 + mb, tfrom[ss, mb], tto[ss, mb]]
                extra = float(np.sum(gmv[ss, mb] * orig, dtype=np.float64))
                gmv[ss, mb] = 0.0
        _HOST_TG_EXTRA.append(extra)

        def pack_idx(bsel):
            # flat index into [T, S, BH, T] layout
            f = tfrom[:, bsel]
            u = tto[:, bsel]
            srange = np.arange(n_steps, dtype=np.int64)[:, None]
            brange = np.arange(BH, dtype=np.int64)[None, :]
            flat = ((f * n_steps + srange) * BH + brange) * T + u
            return np.ascontiguousarray(
                flat.reshape(-1).reshape(NG, 128).T.astype(np.int32)
            )

        def pack_msk(bsel):
            return np.ascontiguousarray(
                gmv[:, bsel].reshape(-1).reshape(NG, 128).T.astype(np.float32)
            )

        lo = slice(0, BH)
        hic = slice(BH, BL)
        in_maps.append(
            {
                "sc16": sc16_c,
                "sc8": sc8_c,
                "p0t": p0_c,
                "g16_idx": pack_idx(lo),
                "g16_msk": pack_msk(lo),
                "g8_idx": pack_idx(hic),
                "g8_msk": pack_msk(hic),
            }
        )
    return in_maps


_HOST_TG_EXTRA = [0.0] * NCORES


def combine(results, n_steps=S):
    """Host-side reduction of per-core partials -> scalar loss."""
    part = 0.0
    tg = 0.0
    for c, r in enumerate(results):
        qf = np.asarray(r["out_q"], dtype=np.float64)
        part += float(np.sum(np.log(qf[END_TAG, :])))
        part += BL * n_steps * C_SHIFT
        tg += float(r["out_tg"].sum(dtype=np.float64))
        tg += _HOST_TG_EXTRA[c] if c < len(_HOST_TG_EXTRA) else 0.0
    return np.float32((part - tg) / B)


_NC_CACHE = {}


def kernel(scores, target, mask_for_gold, mask_for_padding):
    if "nc" not in _NC_CACHE:
        _NC_CACHE["nc"] = build(S)
    nc = _NC_CACHE["nc"]
    in_maps = make_in_maps(scores, target, mask_for_gold, mask_for_padding, S)
    res = bass_utils.run_bass_kernel_spmd(
        nc, in_maps, core_ids=list(range(NCORES))
    )
    return combine(res.results, S)


# revision 34
# speedup vs baseline: 2.2220x; 2.0857x over previous
"""CRF loss (forward-algorithm partition + gold energy) on 8 TRN2 NeuronCores.

Strategy (data-parallel over batch; 64 batches -> 8 cores x 8 local):
  - Linear-domain recurrence per local batch with E = exp(scores - c).
    The forward algorithm is a chain of matrix products applied to a
    vector, and BOTH ends of the chain are known vectors (q0 forward,
    one-hot(END_TAG) backward), so the kernel runs TWO independent
    half-length vector chains concurrently -- forward over steps 1..127
    and backward over steps 255..128 (whose score blocks the host stores
    pre-transposed).  The host combines them with a dot product per
    batch.  This halves the serial PE->PSUM->copy->PE dependency chain
    that bounds the recurrence.  The two chains' per-step PSUM->SBUF
    state copies run on DIFFERENT engines (VectorE for forward, ScalarE
    for backward) so the in-order engine streams cannot serialize the
    chains against each other.
  - A constant shift c = ln(128*sqrt(e)) is folded into every exponential
    so the states stay O(1) for all steps: no renormalization anywhere.
    The host adds 256*c per batch when assembling the loss.
  - exp work is split across THREE engines: batches 0..2 stream as bf16
    and go through a one-instruction Schraudolph exp on VectorE
    (tensor_scalar mult+add producing bf16 bit patterns as int16, 4x
    perf mode, debiased); batches 3..5 stream as fp8e4 through ScalarE's
    exp LUT; batches 6..7 stream as fp8e4 through the same Schraudolph
    on GpSimd.  exp instructions are split into pieces and interleaved
    between the state copies to limit head-of-line blocking on the
    in-order engine queues.  Bytes/elem averages 1.375, cutting HBM
    traffic ~3x vs f32.
  - mask_for_padding is folded into the scores on the host: blocks with
    mask==0 become a log-domain identity (0 diag / -30 off-diag), which
    exponentiates to e^-c * I and passes the state through unchanged up
    to the uniformly-counted shift.  (The harness always passes an
    all-ones padding mask, so this rewrites nothing.)
  - Scores are stored partition-major [tag, step, batch, tag] so every
    chunk DMA is one fully contiguous run per partition.
  - Gold-path energy: two single-instruction indirect-DMA gathers with
    host-precomputed flat indices, masked multiply-reduce on VectorE.
"""

import math
import os

import numpy as np

import concourse.bacc as bacc
import concourse.bass as bass
import concourse.mybir as mybir
import concourse.tile as tile
from concourse import bass_utils

S = 256
B = 64
T = 128
NCORES = 8
BL = B // NCORES  # 8 local batches per core
NB16 = 2  # batches 0..1: bf16, VectorE Schraudolph exp (4x mode)
NB8 = 6  # batches 2..7: fp8 (2,3: VectorE 1x, 4,5: ScalarE LUT, 6,7: GpSimd)
NBACT = 2
START_TAG = 126
END_TAG = 127
CHUNK = 8  # timesteps per score DMA per direction
FSPLIT = 128  # forward chain covers steps 1..FSPLIT-1, backward FSPLIT..S-1

# constant shift folded into every exp so the states stay O(1)
C_SHIFT = math.log(128.0) + 0.5
# Schraudolph exp-as-bf16-bits constants: bits = round(x*A + B)
A_SCH = 128.0 / math.log(2.0)  # 184.6644
# 16256 = 127<<7 (bf16 exponent bias), minus c in bits, minus mean-error debias
B_SCH = 16256.0 - C_SHIFT * A_SCH - (0.0397 / math.log(2.0)) * 128.0

f32 = mybir.dt.float32
bf16 = mybir.dt.bfloat16
fp8 = mybir.dt.float8e4
i16 = mybir.dt.int16
i32 = mybir.dt.int32
Exp = mybir.ActivationFunctionType.Exp
Alu = mybir.AluOpType

NG16 = S * NB16 // 128  # gather columns, bf16 tensor (= 6)
NG8 = S * NB8 // 128  # gather columns, fp8 tensor (= 10)


def build(n_steps=S):
    """Build + compile the SPMD kernel for one core's batch shard."""
    nc = bacc.Bacc(
        "TRN2", target_bir_lowering=False, debug=False, num_devices=NCORES
    )
    # partition-major layouts: [tag, step, batch, tag]
    # (forward-range blocks are [from, .., to]; backward-range transposed)
    sc16 = nc.dram_tensor("sc16", [T, n_steps, NB16, T], bf16,
                          kind="ExternalInput")
    sc8 = nc.dram_tensor("sc8", [T, n_steps, NB8, T], fp8, kind="ExternalInput")
    p0 = nc.dram_tensor("p0t", [T, BL], f32, kind="ExternalInput").ap()
    gi16 = nc.dram_tensor("g16_idx", [128, NG16], i32, kind="ExternalInput").ap()
    gm16 = nc.dram_tensor("g16_msk", [128, NG16], f32, kind="ExternalInput").ap()
    gi8 = nc.dram_tensor("g8_idx", [128, NG8], i32, kind="ExternalInput").ap()
    gm8 = nc.dram_tensor("g8_msk", [128, NG8], f32, kind="ExternalInput").ap()
    zb = nc.dram_tensor("zb0", [T, BL], bf16, kind="ExternalInput").ap()
    o_qf = nc.dram_tensor("out_qf", [T, BL], f32, kind="ExternalOutput").ap()
    o_qb = nc.dram_tensor("out_qb", [T, BL], f32, kind="ExternalOutput").ap()
    o_tg = nc.dram_tensor("out_tg", [128, 1], f32, kind="ExternalOutput").ap()

    with tile.TileContext(nc) as tc:
        _body(nc, tc, sc16, sc8, p0, gi16, gm16, gi8, gm8, zb, o_qf, o_qb,
              o_tg, n_steps)
    nc.compile()
    return nc


def _body(nc, tc, sc16, sc8, p0, gi16, gm16, gi8, gm8, zb, o_qf, o_qb, o_tg,
          n_steps):
    from contextlib import ExitStack

    nogather = os.environ.get("K_NOGATHER")
    repeat = int(os.environ.get("K_REPEAT", "1"))

    sc16_ap = sc16.ap()
    sc8_ap = sc8.ap()

    with ExitStack() as ctx:
        const = ctx.enter_context(tc.tile_pool(name="const", bufs=1))
        s16p = ctx.enter_context(tc.tile_pool(name="s16p", bufs=4))
        s8p = ctx.enter_context(tc.tile_pool(name="s8p", bufs=4))
        epool = ctx.enter_context(tc.tile_pool(name="epool", bufs=3))
        vpool = ctx.enter_context(tc.tile_pool(name="vpool", bufs=4, space="PSUM"))
        small = ctx.enter_context(tc.tile_pool(name="small", bufs=2))

        # ---- persistent state (ping-pong per direction) ----
        stf = [const.tile([128, BL], bf16, name="stf0"),
               const.tile([128, BL], bf16, name="stf1")]
        stb = [const.tile([128, BL], bf16, name="stb0"),
               const.tile([128, BL], bf16, name="stb1")]
        p0_sb = const.tile([128, BL], f32)
        nc.sync.dma_start(out=p0_sb[:], in_=p0[:])
        zb_sb = const.tile([128, BL], bf16)
        nc.sync.dma_start(out=zb_sb[:], in_=zb[:])
        biasc = const.tile([128, 1], f32)
        nc.vector.memset(biasc[:], -C_SHIFT)

        # ---- gold energy gather (independent of the recurrence) ----
        if not nogather:
            gidx16 = const.tile([128, NG16], i32)
            gidx8 = const.tile([128, NG8], i32)
            gmsk = const.tile([128, NG16 + NG8], f32)
            nc.sync.dma_start(out=gidx16[:], in_=gi16[:])
            nc.sync.dma_start(out=gidx8[:], in_=gi8[:])
            nc.sync.dma_start(out=gmsk[:, :NG16], in_=gm16[:])
            nc.sync.dma_start(out=gmsk[:, NG16:], in_=gm8[:])
            gath16 = const.tile([128, NG16], bf16)
            gath8 = const.tile([128, NG8], fp8)
            n16 = T * n_steps * NB16 * T
            n8 = T * n_steps * NB8 * T
            sc16_flat = bass.AP(tensor=sc16, offset=0, ap=[[1, n16], [1, 1]])
            sc8_flat = bass.AP(tensor=sc8, offset=0, ap=[[1, n8], [1, 1]])
            # one gather per column, emitted spread across the chunk loop so
            # they fill GpSimd idle slots instead of delaying its exp pieces
            gather_jobs = [
                (gath16, sc16_flat, gidx16, j) for j in range(NG16)
            ] + [(gath8, sc8_flat, gidx8, j) for j in range(NG8)]
            gcat = const.tile([128, NG16 + NG8], f32)
        else:
            gather_jobs = []
            tgz = const.tile([128, 1], f32)
            nc.vector.memset(tgz[:], 0.0)
            nc.sync.dma_start(out=o_tg[:], in_=tgz[:])

        # ---- the two half-chains over timesteps ----
        EXP_PIECE = 4  # slots covered by one exp instruction piece

        def emit_exp(e, t16, t8, sla, slb):
            # VectorE: batches 0..1 (Schraudolph, bf16 bits via i16, 4x)
            nc.vector.tensor_scalar(
                out=e.bitcast(i16)[:, sla:slb, 0:NB16, :],
                in0=t16[:, sla:slb, :, :],
                scalar1=A_SCH,
                scalar2=B_SCH,
                op0=Alu.mult,
                op1=Alu.add,
            )
            # VectorE: batches 2,3 (Schraudolph from fp8, 1x)
            nc.vector.tensor_scalar(
                out=e.bitcast(i16)[:, sla:slb, NB16 : NB16 + 2, :],
                in0=t8[:, sla:slb, 0:2, :],
                scalar1=A_SCH,
                scalar2=B_SCH,
                op0=Alu.mult,
                op1=Alu.add,
            )
            # ScalarE: batches 4,5 (LUT exp)
            nc.scalar.activation(
                out=e[:, sla:slb, NB16 + 2 : NB16 + 2 + NBACT, :],
                in_=t8[:, sla:slb, 2 : 2 + NBACT, :],
                func=Exp,
                bias=biasc[:],
            )
            # GpSimd: batches 6..7 (Schraudolph)
            nc.gpsimd.tensor_scalar(
                out=e.bitcast(i16)[:, sla:slb, NB16 + 2 + NBACT : BL, :],
                in0=t8[:, sla:slb, 2 + NBACT : NB8, :],
                scalar1=A_SCH,
                scalar2=B_SCH,
                op0=Alu.mult,
                op1=Alu.add,
            )

        for rep in range(repeat):
            last_rep = rep == repeat - 1
            # forward init: a = exp(p0 - c)
            nc.scalar.activation(out=stf[0][:], in_=p0_sb[:], func=Exp,
                                 bias=biasc[:])
            # backward init: z = one-hot(END_TAG), host-uploaded
            nc.vector.tensor_copy(out=stb[0][:], in_=zb_sb[:])

            nfwd = FSPLIT - 1  # steps 1 .. FSPLIT-1
            nchunks = -(-max(nfwd, n_steps - FSPLIT) // CHUNK)
            PREFETCH = 2  # chunks of DMA issued ahead of their compute

            def chunk_ranges(ci):
                fs0 = 1 + CHUNK * ci
                fs1 = min(fs0 + CHUNK, FSPLIT)
                bhi = n_steps - CHUNK * ci
                blo = max(bhi - CHUNK, FSPLIT)
                return fs0, fs1, bhi, blo

            def issue_dmas(ci):
                fs0, fs1, bhi, blo = chunk_ranges(ci)
                tiles = [None] * 4
                if fs1 > fs0:
                    nsf = fs1 - fs0
                    tf16 = s16p.tile([128, nsf, NB16, T], bf16, tag="t16f",
                                     name="t16f")
                    nc.sync.dma_start(out=tf16[:], in_=sc16_ap[:, fs0:fs1])
                    tf8 = s8p.tile([128, nsf, NB8, T], fp8, tag="t8f",
                                   name="t8f")
                    nc.sync.dma_start(out=tf8[:], in_=sc8_ap[:, fs0:fs1])
                    tiles[0], tiles[1] = tf16, tf8
                if bhi > blo:
                    nsb = bhi - blo
                    tb16 = s16p.tile([128, nsb, NB16, T], bf16, tag="t16b",
                                     name="t16b")
                    nc.sync.dma_start(out=tb16[:], in_=sc16_ap[:, blo:bhi])
                    tb8 = s8p.tile([128, nsb, NB8, T], fp8, tag="t8b",
                                   name="t8b")
                    nc.sync.dma_start(out=tb8[:], in_=sc8_ap[:, blo:bhi])
                    tiles[2], tiles[3] = tb16, tb8
                return tiles

            pending = {}
            e_tiles = {}
            issued = 0
            if rep == 0:
                chunk_marks = []

            def alloc_e(ci):
                fs0, fs1, bhi, blo = chunk_ranges(ci)
                ef = eb = None
                if fs1 > fs0:
                    ef = epool.tile([128, fs1 - fs0, BL, T], bf16, tag="ef",
                                    name="ef")
                if bhi > blo:
                    eb = epool.tile([128, bhi - blo, BL, T], bf16, tag="eb",
                                    name="eb")
                e_tiles[ci] = (ef, eb)

            def emit_pieces(ci, k):
                # emit the k-th exp piece pair for chunk ci
                fs0, fs1, bhi, blo = chunk_ranges(ci)
                nsf, nsb = fs1 - fs0, bhi - blo
                ef, eb = e_tiles[ci]
                sl = k * EXP_PIECE
                tf16, tf8, tb16, tb8 = pending[ci]
                if ef is not None and sl < nsf:
                    emit_exp(ef, tf16, tf8, sl, min(sl + EXP_PIECE, nsf))
                if eb is not None and sl < nsb:
                    # backward steps consume tile indices descending: tail first
                    emit_exp(eb, tb16, tb8, max(0, nsb - sl - EXP_PIECE),
                             nsb - sl)

            for ci in range(nchunks):
                while issued <= min(ci + PREFETCH, nchunks - 1):
                    pending[issued] = issue_dmas(issued)
                    issued += 1
                if ci == 0:
                    alloc_e(0)
                    for k in range(-(-CHUNK // EXP_PIECE)):
                        emit_pieces(0, k)
                fs0, fs1, bhi, blo = chunk_ranges(ci)
                nsf = fs1 - fs0
                nsb = bhi - blo
                ef, eb = e_tiles[ci]
                for sl in range(CHUNK):
                    if sl % EXP_PIECE == 0 and ci + 1 < nchunks:
                        k = sl // EXP_PIECE
                        if k == 0:
                            alloc_e(ci + 1)
                        emit_pieces(ci + 1, k)
                    # ---- forward step ----
                    step = fs0 + sl
                    if step < fs1:
                        sin = stf[(step - 1) % 2]
                        sout = stf[step % 2]
                        vf = vpool.tile([128, BL], f32, tag="vf", name="vf")
                        for b in range(BL):
                            nc.tensor.matmul(
                                out=vf[:, b : b + 1],
                                lhsT=ef[:, sl, b, :],
                                rhs=sin[:, b : b + 1],
                                start=True,
                                stop=True,
                            )
                        if step == FSPLIT - 1 and last_rep:
                            qff = small.tile([128, BL], f32, tag="qff",
                                             name="qff")
                            ci_inst = nc.vector.tensor_copy(out=qff[:], in_=vf[:])
                            nc.sync.dma_start(out=o_qf[:], in_=qff[:])
                        else:
                            ci_inst = nc.vector.tensor_copy(out=sout[:], in_=vf[:])
                        if rep == 0 and sl == 0:
                            chunk_marks.append(ci_inst.ins.name)
                    # ---- backward step (state copy on ScalarE) ----
                    bstep = bhi - 1 - sl
                    if bstep >= blo:
                        j = n_steps - bstep  # 1-based backward slot
                        sin = stb[(j - 1) % 2]
                        sout = stb[j % 2]
                        vb = vpool.tile([128, BL], f32, tag="vb", name="vb")
                        for b in range(BL):
                            nc.tensor.matmul(
                                out=vb[:, b : b + 1],
                                lhsT=eb[:, bstep - blo, b, :],
                                rhs=sin[:, b : b + 1],
                                start=True,
                                stop=True,
                            )
                        if bstep == FSPLIT and last_rep:
                            qbf = small.tile([128, BL], f32, tag="qbf",
                                             name="qbf")
                            nc.scalar.copy(out=qbf[:], in_=vb[:])
                            nc.sync.dma_start(out=o_qb[:], in_=qbf[:])
                        else:
                            nc.scalar.copy(out=sout[:], in_=vb[:])

        if not nogather:
            for k, (gt, gflat, gix, j) in enumerate(gather_jobs):
                gi_inst = nc.gpsimd.indirect_dma_start(
                    out=gt[:, j : j + 1],
                    out_offset=None,
                    in_=gflat,
                    in_offset=bass.IndirectOffsetOnAxis(
                        ap=gix[:, j : j + 1], axis=0
                    ),
                )
                if k < len(chunk_marks):
                    gi_inst.ins.add_sync_dependencies_from(
                        bass._bass_rust.InstructionNameOrderedSet([chunk_marks[k]])
                    )
            nc.vector.tensor_copy(out=gcat[:, :NG16], in_=gath16[:])
            nc.vector.tensor_copy(out=gcat[:, NG16:], in_=gath8[:])
            prod = const.tile([128, NG16 + NG8], f32)
            nc.vector.tensor_tensor(
                out=prod[:], in0=gcat[:], in1=gmsk[:], op=Alu.mult
            )
            tgc = const.tile([128, 1], f32)
            nc.vector.reduce_sum(out=tgc[:], in_=prod[:], axis=mybir.AxisListType.X)
            nc.sync.dma_start(out=o_tg[:], in_=tgc[:])


def make_in_maps(scores, target, mask_gold, mask_pad, n_steps=S):
    """Host-side sharding/layout -> per-core input dicts."""
    global _HOST_TG_EXTRA
    scores = np.asarray(scores, dtype=np.float32)
    target = np.asarray(target).astype(np.int64)
    mg = np.asarray(mask_gold).astype(np.float32)
    mp = np.asarray(mask_pad).astype(np.int32)

    bf16_np = mybir.dt.np(bf16)
    fp8_np = mybir.dt.np(fp8)

    any_masked = not bool(np.all(mp[1:n_steps] == 1))
    if any_masked:
        diagpat = np.full((T, T), -30.0, dtype=np.float32)
        np.fill_diagonal(diagpat, 0.0)

    in_maps = []
    _HOST_TG_EXTRA = []
    for c in range(NCORES):
        b0 = c * BL
        blk = np.array(scores[:n_steps, b0 : b0 + BL])  # [S, BL, T, T] copy
        extra = 0.0
        if any_masked:
            ms, mb = np.nonzero(mp[1:n_steps, b0 : b0 + BL] == 0)
            if ms.size:
                blk[ms + 1, mb] = diagpat
        # forward range: [from, step, batch, to]; backward range transposed
        tr = np.empty((T, n_steps, BL, T), dtype=np.float32)
        tr[:, :FSPLIT] = blk[:FSPLIT].transpose(2, 0, 1, 3)
        tr[:, FSPLIT:] = blk[FSPLIT:].transpose(3, 0, 1, 2)
        sc16_c = tr[:, :, 0:NB16, :].astype(bf16_np)
        sc8_c = np.ascontiguousarray(tr[:, :, NB16:BL, :]).astype(fp8_np)
        p0_c = np.ascontiguousarray(scores[0, b0 : b0 + BL, START_TAG, :].T)

        tgt = target[:n_steps, b0 : b0 + BL, 0]
        tfrom = tgt // T
        tto = tgt % T
        gmv = np.array(mg[:n_steps, b0 : b0 + BL])
        if any_masked:
            ms, mb = np.nonzero(mp[1:n_steps, b0 : b0 + BL] == 0)
            if ms.size:
                ss = ms + 1
                orig = scores[ss, b0 + mb, tfrom[ss, mb], tto[ss, mb]]
                extra = float(np.sum(gmv[ss, mb] * orig, dtype=np.float64))
                gmv[ss, mb] = 0.0
        _HOST_TG_EXTRA.append(extra)

        srange = np.arange(n_steps, dtype=np.int64)[:, None]
        # partition tag: 'from' for forward-range steps, 'to' for backward
        ptag = np.where(srange < FSPLIT, tfrom, tto)
        qtag = np.where(srange < FSPLIT, tto, tfrom)

        def pack_idx(bsel, nb, ng):
            brange = np.arange(nb, dtype=np.int64)[None, :]
            flat = ((ptag[:, bsel] * n_steps + srange) * nb + brange) * T + qtag[
                :, bsel
            ]
            return np.ascontiguousarray(
                flat.reshape(-1).reshape(ng, 128).T.astype(np.int32)
            )

        def pack_msk(bsel, ng):
            return np.ascontiguousarray(
                gmv[:, bsel].reshape(-1).reshape(ng, 128).T.astype(np.float32)
            )

        zb0 = np.zeros((T, BL), dtype=bf16_np)
        zb0[END_TAG, :] = 1.0
        lo = slice(0, NB16)
        hic = slice(NB16, BL)
        in_maps.append(
            {
                "sc16": sc16_c,
                "sc8": sc8_c,
                "p0t": p0_c,
                "zb0": zb0,
                "g16_idx": pack_idx(lo, NB16, NG16),
                "g16_msk": pack_msk(lo, NG16),
                "g8_idx": pack_idx(hic, NB8, NG8),
                "g8_msk": pack_msk(hic, NG8),
            }
        )
    return in_maps


_HOST_TG_EXTRA = [0.0] * NCORES


def combine(results, n_steps=S):
    """Host-side reduction of per-core partials -> scalar loss."""
    part = 0.0
    tg = 0.0
    for c, r in enumerate(results):
        a = np.asarray(r["out_qf"], dtype=np.float64)
        z = np.asarray(r["out_qb"], dtype=np.float64)
        dots = np.sum(a * z, axis=0)  # per-batch a . z
        part += float(np.sum(np.log(dots)))
        part += BL * n_steps * C_SHIFT
        tg += float(r["out_tg"].sum(dtype=np.float64))
        tg += _HOST_TG_EXTRA[c] if c < len(_HOST_TG_EXTRA) else 0.0
    return np.float32((part - tg) / B)


_NC_CACHE = {}


def kernel(scores, target, mask_for_gold, mask_for_padding):
    if "nc" not in _NC_CACHE:
        _NC_CACHE["nc"] = build(S)
    nc = _NC_CACHE["nc"]
    in_maps = make_in_maps(scores, target, mask_for_gold, mask_for_padding, S)
    res = bass_utils.run_bass_kernel_spmd(
        nc, in_maps, core_ids=list(range(NCORES))
    )
    return combine(res.results, S)
